# revision 39
# baseline (speedup 1.0000x reference)
"""GQA attention block (B=1, S=2048, D=4096, H=32/HK=8, HD=128, T_CACHE=2048)
tensor-parallel over heads across 8 NeuronCores.

Per core c: q-heads 4c..4c+3, kv-head c. All matmuls in bf16 on the PE
(fp32 accumulate in PSUM); softmax statistics and normalization in fp32.

Layout strategy (contraction dim must live on SBUF partitions):
  - host pre-permutes every DRAM operand to a partition-major layout so
    each DMA is one contiguous run per partition (the naive rearranges
    cost ~67k descriptors and stalled the PE at block boundaries, each
    stall also triggering a ~6.8us HAM re-throttle window)
  - scoresT [t, s] = matmul(lhsT=keysT[hd, t-chunk], rhs=qT[hd, s]);
    two consecutive t-chunks accumulate into one 2-bank PSUM tile and a
    single ACT instruction computes exp over both ([128, 1024]),
    amortizing the ~352-cycle ACT fixed overhead that otherwise paces
    the attention loop above the PE rate
  - probsT = exp(scoresT) * mask01 (mask01 = exp(mask), 0/1 causal,
    precomputed host-side); the multiply runs only on the ~4 diagonal
    chunks per block that actually have partial masks
  - PV: out[s,hd | den] = matmul(lhsT=probsT[t, s-sub], rhs=[vals|1]);
    softmax denominator rides as output column 128. Sub-matmuls whose
    (t-chunk, s-sub) tile is fully masked are skipped. The four PV
    accumulators pack two-per-PSUM-bank ([128, 2, 130] tiles); only the
    first matmul emitted into a bank uses start=True (start clears the
    whole bank's has_written bits).
  - normalize on DVE, PE-transpose to attT [hd, s], per-s-block
    AllGather of the 8 cores' head shards, then each core computes a
    512-wide column shard of out = attn @ wo.

Pipelining: block b-1's wo matmuls are interleaved (3 per chunk-pair)
into block b's score/PV stream as soon as b-1's AllGather lands; the
last block gathers in two head-halves so only ~its own wo chunks drain
serially at the end. AllGather-dependent DMAs ride the gpsimd queue and
output DMAs the scalar queue so they can't head-of-line block the sync
queue's streaming loads.
"""

import os
import sys
import numpy as np

for _p in ("/opt/trn_rl_repo", "/root/.axon_site/_ro/trn_rl_repo"):
    if os.path.isdir(_p) and _p not in sys.path:
        sys.path.append(_p)

import ml_dtypes

import concourse.bass as bass
import concourse.mybir as mybir
import concourse.tile as tile
from concourse import bacc
from concourse.bass import ds, ts
from concourse.bass_utils import run_bass_kernel_spmd

# The default platform flags disable the backend LDWEIGHTS optimization;
# with ~3800 LDW+MM pairs the exposed weight-load time costs ~70us of PE.
# Enable it (correctness is verified against the fp32 reference).
try:
    from concourse.compiler_utils import get_compiler_flags, set_compiler_flags
    set_compiler_flags([
        f.replace("--enable-ldw-opt=false", "--enable-ldw-opt=true")
        for f in get_compiler_flags()
    ])
except Exception:
    pass

BF16 = mybir.dt.bfloat16
F32 = mybir.dt.float32
NPBF16 = ml_dtypes.bfloat16

N_CORES = 8
S = 2048
D = 4096
HD = 128
H = 32
HK = 8
T_CACHE = 2048
T = T_CACHE + S
NH = H // N_CORES          # q heads per core
SB = 512                   # s-block
NB = S // SB               # 4 s-blocks
NJ = T // 128              # 32 t-chunks
NJ_CACHE = T_CACHE // 128  # 16
KD = D // 128              # 32 contraction chunks over D
P = 128

_BUILD_CACHE = {}


def build_kernel(nj_active, mask_from, mask_rows, skip_map, mul_map):
    """nj_active[b]: number of t-chunks attended by s-block b (prefix of
    0..NJ). mask_from: first t-chunk index needing the mask01 multiply.
    skip_map[b]: frozenset of (j, m) whose (t-chunk, s-sub) tile is fully
    masked (PV sub-matmul skipped). mul_map[b]: frozenset of j whose mask
    tile is not all-ones (multiply needed)."""
    key = (tuple(nj_active), mask_from, mask_rows,
           tuple(tuple(sorted(s)) for s in skip_map),
           tuple(tuple(sorted(s)) for s in mul_map))
    if key in _BUILD_CACHE:
        return _BUILD_CACHE[key]

    n_mask_chunks = NJ - mask_from
    # per (b, j): number of leading fully-masked s-subs -> the scores
    # matmul can skip those columns. The stale PSUM left there holds
    # earlier (bounded) scores, so its exp stays finite and the mask01
    # multiply zeroes it before PV ever reads it.
    s_lo = []
    for b in range(NB):
        d = {}
        for j in range(nj_active[b]):
            m = 0
            while m < NB and (j, m) in skip_map[b]:
                m += 1
            if 0 < m < NB:
                d[j] = m * P
        s_lo.append(d)

    nc = bacc.Bacc("TRN2", target_bir_lowering=False, debug=False,
                   num_devices=N_CORES)

    xT_e = nc.dram_tensor("xT", [P, NB, KD, SB], BF16, kind="ExternalInput")
    wq_e = nc.dram_tensor("wq", [P, KD, NH * HD], BF16, kind="ExternalInput")
    wk_e = nc.dram_tensor("wk", [P, KD, HD], BF16, kind="ExternalInput")
    wv_e = nc.dram_tensor("wv", [P, KD, HD], BF16, kind="ExternalInput")
    wo_e = nc.dram_tensor("wo", [P, KD, SB], BF16, kind="ExternalInput")
    ckT_e = nc.dram_tensor("ckT", [HD, T_CACHE], BF16, kind="ExternalInput")
    cv_e = nc.dram_tensor("cv", [P, NJ_CACHE, HD], BF16, kind="ExternalInput")
    ropes_e = nc.dram_tensor("ropes", [HD, S], F32, kind="ExternalInput")
    ropep_e = nc.dram_tensor("ropep", [HD, S], F32, kind="ExternalInput")
    maskT_e = nc.dram_tensor("maskT", [P, NB, n_mask_chunks, SB], BF16,
                             kind="ExternalInput")
    out_e = nc.dram_tensor("out", [S, SB], F32, kind="ExternalOutput")

    with tile.TileContext(nc) as tc:
        with (
            tc.tile_pool(name="persist", bufs=1) as persist,
            tc.tile_pool(name="dram", bufs=1, space="DRAM") as dram,
        ):
            keysT = persist.tile([P, T], BF16)
            vals = persist.tile([P, NJ, HD + 1], BF16)
            qT = persist.tile([P, NH, S], BF16)
            ident = persist.tile([P, P], BF16)
            warm = persist.tile([P, 1], BF16)

            attT_my = [
                dram.tile([P, NH * SB], BF16, name=f"agin{b}")
                for b in range(NB - 1)
            ]
            attT_all = [
                dram.tile([N_CORES * P, NH * SB], BF16, name=f"agout{b}",
                          addr_space="Shared")
                for b in range(NB - 1)
            ]
            # first and last blocks gather in two head-halves (earlier
            # first-half availability / shorter final exposure)
            attT_myh = {
                (b, i): dram.tile([P, 2 * SB], BF16, name=f"aginh{b}_{i}")
                for b in (0, NB - 1) for i in range(2)
            }
            attT_allh = {
                (b, i): dram.tile([N_CORES * P, 2 * SB], BF16,
                                  name=f"agouth{b}_{i}", addr_space="Shared")
                for b in (0, NB - 1) for i in range(2)
            }

            # skew sponge: a tiny AllGather issued first thing. Collectives
            # on the ring serialize, and per-core launch skew (observed up
            # to ~70us) is absorbed by whichever collective runs first; this
            # one completes under stage A's projections instead of putting
            # that skew on the first attT gather's critical path.
            sponge_in = dram.tile([1, 16], BF16, name="sponge_in")
            sponge_out = dram.tile([N_CORES, 16], BF16, name="sponge_out",
                                   addr_space="Shared")
            nc.gpsimd.collective_compute(
                "AllGather",
                mybir.AluOpType.bypass,
                replica_groups=[list(range(N_CORES))],
                ins=[sponge_in.opt()],
                outs=[sponge_out.opt()],
            )

            nc.gpsimd.memset(ident, 0.0)
            nc.gpsimd.affine_select(
                out=ident, in_=ident, compare_op=mybir.AluOpType.not_equal,
                fill=1.0, base=0, pattern=[[-1, P]], channel_multiplier=1,
            )
            # pre-warm the ACT exp table set during stage A
            nc.scalar.activation(warm, ident[:, 0:1],
                                 mybir.ActivationFunctionType.Exp)
            nc.vector.memset(vals[:, :, HD : HD + 1], 1.0)

            # ---- stage A: projections ----
            with (
                tc.tile_pool(name="wA", bufs=1) as wA,
                tc.tile_pool(name="xt", bufs=2) as xtp,
                tc.tile_pool(name="psA", bufs=6, space="PSUM") as psA,
            ):
                wq_sb = wA.tile([P, KD, NH * HD], BF16)
                wk_sb = wA.tile([P, KD, HD], BF16)
                wv_sb = wA.tile([P, KD, HD], BF16)
                ropes_sb = wA.tile([P, S], F32)
                ropep_sb = wA.tile([P, S], F32)

                # first-needed chunks first: the k=0 weight piece and the
                # first xt chunks gate the very first matmul
                xt0 = xtp.tile([P, KD, SB], BF16, tag="xt", name="xt0")
                # k-granular first group so the very first matmul's inputs
                # arrive as early as possible
                for k in range(4):
                    nc.sync.dma_start(wq_sb[:, ts(k, 1), :],
                                      wq_e.ap()[:, ts(k, 1), :])
                    nc.sync.dma_start(xt0[:, ts(k, 1), :],
                                      xT_e.ap()[:, 0, ts(k, 1), :])
                for g in range(1, 8):
                    nc.sync.dma_start(wq_sb[:, ts(g, 4), :],
                                      wq_e.ap()[:, ts(g, 4), :])
                    nc.sync.dma_start(xt0[:, ts(g, 4), :],
                                      xT_e.ap()[:, 0, ts(g, 4), :])
                for g in range(4):
                    nc.sync.dma_start(wk_sb[:, ts(g, 8), :],
                                      wk_e.ap()[:, ts(g, 8), :])
                    nc.sync.dma_start(wv_sb[:, ts(g, 8), :],
                                      wv_e.ap()[:, ts(g, 8), :])
                nc.sync.dma_start(ropes_sb, ropes_e.ap())
                nc.sync.dma_start(ropep_sb, ropep_e.ap())
                # cache tensors gate stage-B start (~220us). Issued here they
                # finish by ~150us; pushing them later makes the wo load
                # collide with the first AllGather's transfer window, which
                # measurably doubles that collective's latency.
                nc.sync.dma_start(vals[:, 0:NJ_CACHE, 0:HD], cv_e.ap())
                nc.sync.dma_start(keysT[:, 0:T_CACHE], ckT_e.ap())

                for b in range(NB):
                    if b == 0:
                        xt = xt0
                    else:
                        xt = xtp.tile([P, KD, SB], BF16, tag="xt",
                                      name=f"xt{b}")
                        for g in range(4):
                            nc.sync.dma_start(xt[:, ts(g, 8), :],
                                              xT_e.ap()[:, b, ts(g, 8), :])
                    psq = [
                        psA.tile([P, SB], F32, tag="psA", name=f"psq{m}")
                        for m in range(NH)
                    ]
                    for k in range(KD):
                        for m in range(NH):
                            nc.tensor.matmul(
                                psq[m], wq_sb[:, k, ts(m, HD)], xt[:, k, :],
                                start=(k == 0), stop=(k == KD - 1),
                            )
                    for m in range(NH):
                        nc.vector.tensor_mul(
                            out=qT[:, m, ds(b * SB, SB)], in0=psq[m],
                            in1=ropes_sb[:, ds(b * SB, SB)],
                        )
                    psk = psA.tile([P, SB], F32, tag="psA")
                    for k in range(KD):
                        nc.tensor.matmul(
                            psk, wk_sb[:, k, :], xt[:, k, :],
                            start=(k == 0), stop=(k == KD - 1),
                        )
                    nc.vector.tensor_mul(
                        out=keysT[:, ds(T_CACHE + b * SB, SB)], in0=psk,
                        in1=ropep_sb[:, ds(b * SB, SB)],
                    )
                    for m in range(NB):
                        psv = psA.tile([P, SB], F32, tag="psA")
                        for k in range(KD):
                            nc.tensor.matmul(
                                psv[:, 0:HD], xt[:, k, ts(m, P)], wv_sb[:, k, :],
                                start=(k == 0), stop=(k == KD - 1),
                            )
                        nc.any.tensor_copy(
                            vals[:, NJ_CACHE + NB * b + m, 0:HD], psv[:, 0:HD]
                        )
            # ---- stages B+C interleaved ----
            with tc.tile_pool(name="woP", bufs=1) as woP:
                wo_sb = woP.tile([P, KD, SB], BF16)

                with (
                    tc.tile_pool(name="maskp", bufs=2) as maskp,
                    tc.tile_pool(name="ptp", bufs=4) as ptp,
                    tc.tile_pool(name="attsb", bufs=2) as attp,
                    tc.tile_pool(name="attL", bufs=2) as attL,
                    tc.tile_pool(name="small", bufs=8) as small,
                    tc.tile_pool(name="outp", bufs=2) as outp,
                    tc.tile_pool(name="psS", bufs=2, space="PSUM") as psS,
                    tc.tile_pool(name="psPV", bufs=2, space="PSUM") as psPV,
                    tc.tile_pool(name="psTr", bufs=1, space="PSUM") as psTr,
                    tc.tile_pool(name="psO", bufs=1, space="PSUM") as psO,
                ):
                    al_tiles = {}

                    # --- stage C work queue: one wo-matmul at a time ---
                    c_state = {"queue": [], "cur": None, "k": 0,
                               "drain": False, "alt": 0}
                    # blocks whose attT arrives in two head-half gathers
                    # accumulate their wo matmuls first-half-heads first
                    k_last = [4 * c + l for l in (0, 1) for c in range(N_CORES)]
                    k_last += [4 * c + l for l in (2, 3) for c in range(N_CORES)]

                    # queue items are (b, m, lo, hi): positions [lo, hi) of
                    # the block's k-order. The last block's chunks split in
                    # two halves so half0 (whose AllGather lands mid-block)
                    # can be absorbed during h3; the halves meet via an
                    # SBUF partial sum.
                    partials = {}

                    def c_open(item):
                        b, m, lo, hi = item
                        # outside attention (drain), alternate accumulators
                        # between the psO bank and a (now idle) psS bank so
                        # consecutive chunks overlap
                        c_state["alt"] += 1
                        if c_state["drain"] and (c_state["alt"] % 2 == 1):
                            po = psS.tile([P, 2, SB], F32, tag="psS",
                                          name=f"po{b}_{m}_{lo}")[:, 0, :]
                        else:
                            po = psO.tile([P, SB], F32, tag="psO",
                                          name=f"po{b}_{m}_{lo}")
                        c_state["cur"] = (b, m, lo, hi, po)
                        c_state["k"] = lo

                    def c_step(anchor=None):
                        """Emit one pending stage-C matmul, if any. `anchor`
                        pins the matmul's schedule position: the Tile
                        scheduler's cost model treats collectives as instant
                        and would otherwise hoist these wo-matmuls (which
                        transitively wait on an AllGather) into earlier
                        blocks, hard-stalling the in-order PE stream."""
                        if c_state["cur"] is None:
                            if not c_state["queue"]:
                                return False
                            c_open(c_state["queue"].pop(0))
                        b, m, lo, hi, po = c_state["cur"]
                        ki = c_state["k"]
                        k = k_last[ki] if b == NB - 1 else ki
                        cmm = nc.tensor.matmul(
                            po, al_tiles[b][:, k, ts(m, P)], wo_sb[:, k, :],
                            start=(ki == lo), stop=(ki == hi - 1),
                        )
                        if anchor is not None:
                            tile.add_dep_helper(cmm.ins, anchor,
                                                False, "pin C-mm to block")
                        c_state["k"] = ki + 1
                        if c_state["k"] == hi:
                            if lo == 0 and hi < KD:
                                # first half: stash the partial sum in SBUF
                                part = outp.tile([P, SB], F32, tag="part",
                                                 name=f"part{m}", bufs=4)
                                nc.vector.tensor_copy(part, po)
                                partials[(b, m)] = part
                            else:
                                ot = outp.tile([P, SB], F32, tag="outp")
                                if lo == 0:
                                    nc.vector.tensor_copy(ot, po)
                                else:
                                    nc.vector.tensor_add(
                                        out=ot, in0=partials.pop((b, m)),
                                        in1=po,
                                    )
                                nc.scalar.dma_start(
                                    out_e.ap().rearrange(
                                        "(r p) n -> p r n", p=P
                                    )[:, NB * b + m, :],
                                    ot,
                                )
                            c_state["cur"] = None
                        return True

                    def c_drain():
                        while c_step():
                            pass

                    def emit_half_ag(b, attT_sb, half):
                        """Gather heads [2*half, 2*half+2) of block b from
                        all cores, landing them at their global k-chunk
                        positions in the block's al tile."""
                        nc.gpsimd.dma_start(
                            attT_myh[b, half].rearrange("p (h s) -> p h s",
                                                        s=SB),
                            attT_sb[:, 2 * half : 2 * half + 2, :],
                        )
                        nc.gpsimd.collective_compute(
                            "AllGather",
                            mybir.AluOpType.bypass,
                            replica_groups=[list(range(N_CORES))],
                            ins=[attT_myh[b, half].opt()],
                            outs=[attT_allh[b, half].opt()],
                        )
                        if b not in al_tiles:
                            al_tiles[b] = attL.tile(
                                [P, KD, SB], BF16, tag="attL", name=f"alh{b}"
                            )
                        alb = al_tiles[b]
                        allh = attT_allh[b, half].rearrange(
                            "(c p) n -> p c n", p=P
                        )
                        for c in range(N_CORES):
                            nc.gpsimd.dma_start(
                                alb[:, ds(4 * c + 2 * half, 2), :],
                                allh[:, c, :].rearrange("p (l s) -> p l s",
                                                        s=SB),
                            )

                    mask_tiles = {}
                    ag_pending = {}

                    def load_mask(b):
                        nmask = max(0, nj_active[b] - mask_from)
                        if nmask == 0:
                            return
                        mt = maskp.tile([P, n_mask_chunks, SB], BF16,
                                        tag="mt", name=f"mt{b}")
                        nc.sync.dma_start(
                            mt[:, 0:nmask, :],
                            maskT_e.ap()[:, b, 0:nmask, :],
                        )
                        mask_tiles[b] = mt

                    load_mask(0)
                    # wo is needed only once stage-C matmuls start (mid
                    # block 1); issue after the more urgent mask tile
                    for g in range(8):
                        nc.sync.dma_start(wo_sb[:, ts(g, 4), :],
                                          wo_e.ap()[:, ts(g, 4), :])

                    def load_al(bb):
                        al = attL.tile([P, KD, SB], BF16, tag="attL",
                                       name=f"al{bb}")
                        allr = attT_all[bb].rearrange("(c p) n -> p c n", p=P)
                        for c in range(N_CORES):
                            nc.gpsimd.dma_start(
                                al[:, ts(c, NB), :],
                                allr[:, c, :].rearrange("p (h s) -> p h s",
                                                        s=SB),
                            )
                        al_tiles[bb] = al

                    # PSUM start=True clears has_written for the WHOLE bank
                    # (probe-verified), and two PV accumulators share each
                    # psPV bank. The Tile scheduler may freely reorder
                    # writes to disjoint regions, so the bank-clearing
                    # matmul needs explicit scheduling deps: it must come
                    # after the previous h-iteration's partner-region
                    # matmuls, and the partner's first (start=False) write
                    # must come after the clear.
                    pv_prev_last = {}

                    for b in range(NB):
                        nj = nj_active[b]
                        np_pairs = nj // 2
                        # masked (diagonal) pairs FIRST: their DVE mask
                        # multiplies then run at the start of each h when
                        # DVE is idle, instead of colliding with the
                        # normalize chain that gates the next h's PV
                        # accumulators
                        if np_pairs > 2:
                            po_list = [np_pairs - 2, np_pairs - 1]
                            po_list += list(range(np_pairs - 2))
                        else:
                            po_list = list(range(np_pairs))
                        ord_chunks = [c for p in po_list
                                      for c in (2 * p, 2 * p + 1)]
                        first_emit = {}
                        last_emit = {}
                        for m in range(NB):
                            live = [j for j in ord_chunks
                                    if (j, m) not in skip_map[b]]
                            first_emit[m] = live[0]
                            last_emit[m] = live[-1]
                        mt = mask_tiles.get(b)
                        last = b == NB - 1
                        attT_sb = attp.tile([P, NH, SB], BF16, tag="attsb")
                        for h in range(NH):
                            cur_pt = {}
                            pv01 = psPV.tile([P, 2, 130], F32, tag="psPV",
                                             name="pv01")
                            pv23 = psPV.tile([P, 2, 130], F32, tag="psPV",
                                             name="pv23")
                            pv_slice = [pv01[:, 0, 0:129], pv01[:, 1, 0:129],
                                        pv23[:, 0, 0:129], pv23[:, 1, 0:129]]
                            bank_clear = [None, None]  # per psPV bank
                            pv_last = {}

                            def emit_pv(j):
                                for m in range(NB):
                                    if (j, m) in skip_map[b]:
                                        continue
                                    st = False
                                    bank = m // 2
                                    if j == first_emit[m]:
                                        st = bank_clear[bank] is None
                                    mm = nc.tensor.matmul(
                                        pv_slice[m], cur_pt[j][:, ts(m, P)],
                                        vals[:, j, :],
                                        start=st, stop=(j == last_emit[m]),
                                    )
                                    if st:
                                        bank_clear[bank] = mm
                                        # the clear must not hoist between
                                        # the previous iteration's same-bank
                                        # matmuls
                                        for pm in (2 * bank, 2 * bank + 1):
                                            pl = pv_prev_last.get(pm)
                                            if pl is not None:
                                                tile.add_dep_helper(
                                                    mm.ins, pl.ins, False,
                                                    "bank clear after prev")
                                    elif j == first_emit[m]:
                                        # first write of the bank's partner:
                                        # must land after the bank clear
                                        tile.add_dep_helper(
                                            mm.ins, bank_clear[bank].ins,
                                            False, "first write after clear")
                                    pv_last[m] = mm

                            def emit_pv_pair(p):
                                emit_pv(2 * p)
                                emit_pv(2 * p + 1)

                            # arm each block's wo work only once its
                            # AllGather has had generous latency budget
                            # (collective latency has been observed jittering
                            # 17-72us): block 0's wo starts at block 2, not
                            # block 1
                            if b == 2 and h == 0:
                                c_state["queue"].extend(
                                    (0, m, 0, KD) for m in range(NB)
                                )
                            if b >= 2 and h == 2:
                                c_state["queue"].extend(
                                    (b - 1, m, 0, KD) for m in range(NB)
                                )
                            for idx, p in enumerate(po_list):
                                j0, j1 = 2 * p, 2 * p + 1
                                ps = psS.tile([P, 2, SB], F32, tag="psS")
                                smm = None
                                for jj, j in enumerate((j0, j1)):
                                    slo = s_lo[b].get(j, 0)
                                    if b == 0 and h == 0:
                                        # PSUM here may still hold unbounded
                                        # initial garbage whose exp overflows;
                                        # full-width scores overwrite it
                                        slo = 0
                                    mm = nc.tensor.matmul(
                                        ps[:, jj, ds(slo, SB - slo)],
                                        keysT[:, ts(j, P)],
                                        qT[:, h, ds(b * SB + slo, SB - slo)],
                                        start=True, stop=True,
                                    )
                                    if smm is None:
                                        smm = mm
                                pt = ptp.tile([P, 2, SB], BF16, tag="pt")
                                cur_pt[j0] = pt[:, 0, :]
                                cur_pt[j1] = pt[:, 1, :]
                                nc.scalar.activation(
                                    pt, ps, mybir.ActivationFunctionType.Exp
                                )
                                for j in (j0, j1):
                                    if j >= mask_from and j in mul_map[b]:
                                        nc.vector.tensor_mul(
                                            out=cur_pt[j], in0=cur_pt[j],
                                            in1=mt[:, j - mask_from, :],
                                        )
                                # two-pair lag: the first PV of this h lands
                                # two pairs in, giving the previous h's
                                # normalize chain time to release the shared
                                # PV banks
                                if idx >= 2:
                                    emit_pv_pair(po_list[idx - 2])
                                n_pace = (3, 3, 4, 5)[b]
                                for _ in range(n_pace):
                                    c_step(smm.ins)
                            for tp in po_list[-2:]:
                                emit_pv_pair(tp)
                            pv_prev_last = pv_last

                            for m in range(NB):
                                rc = small.tile([P, 1], F32, tag="rc")
                                nc.vector.reciprocal(
                                    rc, pv_slice[m][:, HD : HD + 1]
                                )
                                at = small.tile([P, P], BF16, tag="at")
                                nc.vector.tensor_scalar_mul(
                                    at, pv_slice[m][:, 0:HD], rc
                                )
                                ptr = psTr.tile([P, P], F32, tag="ptr")
                                # transpose as a REGULAR matmul (at.T @ I):
                                # is_transpose matmuls are serialized against
                                # collectives by Tile, which would stall each
                                # block's normalize on the previous AllGather
                                nc.tensor.matmul(ptr, at, ident,
                                                 start=True, stop=True)
                                nc.vector.tensor_copy(
                                    attT_sb[:, h, ts(m, P)], ptr
                                )

                            # last block: gather each half of the heads as
                            # soon as it is done, shortening the final AG
                            # exposure
                            if last and h == 1:
                                emit_half_ag(b, attT_sb, 0)
                            if last and h == 2:
                                # half0's gather landed mid-block: absorb its
                                # wo matmuls during h3
                                c_state["queue"].extend(
                                    (NB - 1, m, 0, KD // 2) for m in range(NB)
                                )
                            if last and h == 3:
                                emit_half_ag(b, attT_sb, 1)
                                c_state["queue"].extend(
                                    (NB - 1, m, KD // 2, KD)
                                    for m in range(NB)
                                )
                            # load the previous block's gathered attT as
                            # early as possible (the unpack DMAs just wait
                            # on the collective in the gpsimd queue, which
                            # has nothing else to do mid-block)
                            if h == 0 and b - 1 in ag_pending:
                                load_al(ag_pending.pop(b - 1))

                        if b + 1 < NB:
                            load_mask(b + 1)
                        if not last:
                            nc.gpsimd.dma_start(
                                attT_my[b].rearrange("p (h s) -> p h s", s=SB),
                                attT_sb,
                            )
                            nc.gpsimd.collective_compute(
                                "AllGather",
                                mybir.AluOpType.bypass,
                                replica_groups=[list(range(N_CORES))],
                                ins=[attT_my[b].opt()],
                                outs=[attT_all[b].opt()],
                            )
                            ag_pending[b] = b

                    c_state["drain"] = True
                    c_drain()

    nc.compile()
    _BUILD_CACHE[key] = nc
    return nc


def _prep_inputs(x, rope, mask, cache_k, cache_v, wq, wk, wv, wo):
    """Host-side shard + partition-major layout prep (every DMA becomes a
    single contiguous run per partition)."""
    scale = np.float32(1.0 / np.sqrt(HD))
    x2 = np.ascontiguousarray(np.asarray(x).reshape(S, D), dtype=np.float32)
    xT = x2.T.astype(NPBF16)                       # [D, S]
    xTh = np.ascontiguousarray(
        xT.reshape(KD, P, NB, SB).transpose(1, 2, 0, 3)
    )                                              # [P, NB, KD, SB]
    rope2 = np.asarray(rope).reshape(S, HD).astype(np.float32)
    ropesT = np.ascontiguousarray((rope2 * scale).T)
    ropepT = np.ascontiguousarray(rope2.T)

    m2 = np.asarray(mask).reshape(S, T).astype(np.float32)
    cache_zero = bool(np.all(m2[:, :T_CACHE] == 0.0))
    causal = m2[:, T_CACHE:]
    # s-block b may skip t-chunk j (j >= 16) iff every entry of the
    # (s-block, chunk) tile is <= -1e3 (exp underflows to ~0 exactly as in
    # the reference softmax).
    nj_active = []
    for b in range(NB):
        nj = NJ
        for j in range(NJ - 1, NJ_CACHE - 1, -1):
            blk = causal[
                b * SB : (b + 1) * SB,
                (j - NJ_CACHE) * 128 : (j - NJ_CACHE + 1) * 128,
            ]
            if np.all(blk <= -1e3):
                nj = j
            else:
                break
        nj_active.append(nj)

    if cache_zero:
        mask_from = NJ_CACHE
        mask_used = causal
    else:
        mask_from = 0
        nj_active = [NJ] * NB
        mask_used = m2
    # keep pairs of t-chunks intact for the paired exp
    nj_active = [min(NJ, nj + (nj % 2)) for nj in nj_active]
    # multiplicative form: probs = exp(scores) * exp(mask)
    mask01 = np.exp(mask_used.astype(np.float64)).astype(NPBF16)
    maskT = np.ascontiguousarray(mask01.T)         # [mask_rows, S]
    mask_rows = maskT.shape[0]
    n_mask_chunks = mask_rows // 128
    maskTh = np.ascontiguousarray(
        maskT.reshape(n_mask_chunks, P, NB, SB).transpose(1, 2, 0, 3)
    )                                              # [P, NB, n_chunks, SB]

    # PV sub-matmul skip map and mask-multiply map, from the actual mask
    skip_map = []
    mul_map = []
    for b in range(NB):
        skips = set()
        muls = set()
        for j in range(mask_from, nj_active[b]):
            # mtile: [128 t, 512 s] (maskT layout: rows=t, cols=s)
            mtile = maskT[
                (j - mask_from) * 128 : (j - mask_from + 1) * 128,
                b * SB : (b + 1) * SB,
            ]
            if not np.all(mtile == np.float32(1.0)):
                muls.add(j)
            for m in range(NB):
                sub = mtile[:, m * P : (m + 1) * P]
                if np.all(sub == np.float32(0.0)):
                    skips.add((j, m))
        skip_map.append(frozenset(skips))
        mul_map.append(frozenset(muls))

    wq_n = np.asarray(wq)
    wk_n = np.asarray(wk)
    wv_n = np.asarray(wv)
    wo_n = np.asarray(wo)
    ck_n = np.asarray(cache_k)
    cv_n = np.asarray(cache_v)

    def pmajor(w, ncols):  # [D, ncols] -> [P, KD, ncols]
        return np.ascontiguousarray(
            w.astype(NPBF16).reshape(KD, P, ncols).transpose(1, 0, 2)
        )

    in_maps = []
    for c in range(N_CORES):
        cvp = np.ascontiguousarray(
            cv_n[0, :, c, :].astype(NPBF16)
            .reshape(NJ_CACHE, P, HD).transpose(1, 0, 2)
        )                                          # [P, NJ_CACHE, HD]
        in_maps.append({
            "xT": xTh,
            "wq": pmajor(wq_n[:, c * NH * HD : (c + 1) * NH * HD], NH * HD),
            "wk": pmajor(wk_n[:, c * HD : (c + 1) * HD], HD),
            "wv": pmajor(wv_n[:, c * HD : (c + 1) * HD], HD),
            "wo": pmajor(wo_n[:, c * SB : (c + 1) * SB], SB),
            "ckT": np.ascontiguousarray(ck_n[0, :, c, :].T).astype(NPBF16),
            "cv": cvp,
            "ropes": ropesT,
            "ropep": ropepT,
            "maskT": maskTh,
        })
    return in_maps, nj_active, mask_from, mask_rows, skip_map, mul_map


def kernel_impl(inputs, trace=False, tmpdir=None):
    in_maps, nj_active, mask_from, mask_rows, skip_map, mul_map = \
        _prep_inputs(**inputs)
    nc = build_kernel(nj_active, mask_from, mask_rows, skip_map, mul_map)
    res = run_bass_kernel_spmd(
        nc, in_maps, core_ids=list(range(N_CORES)), trace=trace, tmpdir=tmpdir
    )
    out = np.concatenate(
        [res.results[c]["out"] for c in range(N_CORES)], axis=1
    ).reshape(1, S, H * HD)
    return np.ascontiguousarray(out, dtype=np.float32), res


def kernel(**inputs) -> np.ndarray:
    out, _ = kernel_impl(inputs, trace=False)
    return out


# revision 42
# speedup vs baseline: 1.0687x; 1.0687x over previous
"""GQA attention block (B=1, S=2048, D=4096, H=32/HK=8, HD=128, T_CACHE=2048)
tensor-parallel over heads across 8 NeuronCores.

Per core c: q-heads 4c..4c+3, kv-head c. All matmuls in bf16 on the PE
(fp32 accumulate in PSUM); softmax statistics and normalization in fp32.

Layout strategy (contraction dim must live on SBUF partitions):
  - host pre-permutes every DRAM operand to a partition-major layout so
    each DMA is one contiguous run per partition (the naive rearranges
    cost ~67k descriptors and stalled the PE at block boundaries, each
    stall also triggering a ~6.8us HAM re-throttle window)
  - scoresT [t, s] = matmul(lhsT=keysT[hd, t-chunk], rhs=qT[hd, s]);
    two consecutive t-chunks accumulate into one 2-bank PSUM tile and a
    single ACT instruction computes exp over both ([128, 1024]),
    amortizing the ~352-cycle ACT fixed overhead that otherwise paces
    the attention loop above the PE rate
  - probsT = exp(scoresT) * mask01 (mask01 = exp(mask), 0/1 causal,
    precomputed host-side); the multiply runs only on the ~4 diagonal
    chunks per block that actually have partial masks
  - PV: out[s,hd | den] = matmul(lhsT=probsT[t, s-sub], rhs=[vals|1]);
    softmax denominator rides as output column 128. Sub-matmuls whose
    (t-chunk, s-sub) tile is fully masked are skipped. The four PV
    accumulators pack two-per-PSUM-bank ([128, 2, 130] tiles); only the
    first matmul emitted into a bank uses start=True (start clears the
    whole bank's has_written bits).
  - normalize on DVE, PE-transpose to attT [hd, s], per-s-block
    AllGather of the 8 cores' head shards, then each core computes a
    512-wide column shard of out = attn @ wo.

Pipelining: block b-1's wo matmuls are interleaved (3 per chunk-pair)
into block b's score/PV stream as soon as b-1's AllGather lands; the
last block gathers in two head-halves so only ~its own wo chunks drain
serially at the end. AllGather-dependent DMAs ride the gpsimd queue and
output DMAs the scalar queue so they can't head-of-line block the sync
queue's streaming loads.
"""

import os
import sys
import numpy as np

for _p in ("/opt/trn_rl_repo", "/root/.axon_site/_ro/trn_rl_repo"):
    if os.path.isdir(_p) and _p not in sys.path:
        sys.path.append(_p)

import ml_dtypes

import concourse.bass as bass
import concourse.mybir as mybir
import concourse.tile as tile
from concourse import bacc
from concourse.bass import ds, ts
from concourse.bass_utils import run_bass_kernel_spmd

# The default platform flags disable the backend LDWEIGHTS optimization;
# with ~3800 LDW+MM pairs the exposed weight-load time costs ~70us of PE.
# Enable it (correctness is verified against the fp32 reference).
try:
    from concourse.compiler_utils import get_compiler_flags, set_compiler_flags
    set_compiler_flags([
        f.replace("--enable-ldw-opt=false", "--enable-ldw-opt=true")
        for f in get_compiler_flags()
    ])
except Exception:
    pass

BF16 = mybir.dt.bfloat16
F32 = mybir.dt.float32
NPBF16 = ml_dtypes.bfloat16

N_CORES = 8
S = 2048
D = 4096
HD = 128
H = 32
HK = 8
T_CACHE = 2048
T = T_CACHE + S
NH = H // N_CORES          # q heads per core
SB = 512                   # s-block
NB = S // SB               # 4 s-blocks
NJ = T // 128              # 32 t-chunks
NJ_CACHE = T_CACHE // 128  # 16
KD = D // 128              # 32 contraction chunks over D
P = 128

_BUILD_CACHE = {}


def build_kernel(nj_active, mask_from, mask_rows, skip_map, mul_map):
    """nj_active[b]: number of t-chunks attended by s-block b (prefix of
    0..NJ). mask_from: first t-chunk index needing the mask01 multiply.
    skip_map[b]: frozenset of (j, m) whose (t-chunk, s-sub) tile is fully
    masked (PV sub-matmul skipped). mul_map[b]: frozenset of j whose mask
    tile is not all-ones (multiply needed)."""
    key = (tuple(nj_active), mask_from, mask_rows,
           tuple(tuple(sorted(s)) for s in skip_map),
           tuple(tuple(sorted(s)) for s in mul_map))
    if key in _BUILD_CACHE:
        return _BUILD_CACHE[key]

    n_mask_chunks = NJ - mask_from
    # per (b, j): number of leading fully-masked s-subs -> the scores
    # matmul can skip those columns. The stale PSUM left there holds
    # earlier (bounded) scores, so its exp stays finite and the mask01
    # multiply zeroes it before PV ever reads it.
    s_lo = []
    for b in range(NB):
        d = {}
        for j in range(nj_active[b]):
            m = 0
            while m < NB and (j, m) in skip_map[b]:
                m += 1
            if 0 < m < NB:
                d[j] = m * P
        s_lo.append(d)

    nc = bacc.Bacc("TRN2", target_bir_lowering=False, debug=False,
                   num_devices=N_CORES)

    xT_e = nc.dram_tensor("xT", [P, NB, KD, SB], BF16, kind="ExternalInput")
    wq_e = nc.dram_tensor("wq", [P, KD, NH * HD], BF16, kind="ExternalInput")
    wk_e = nc.dram_tensor("wk", [P, KD, HD], BF16, kind="ExternalInput")
    wv_e = nc.dram_tensor("wv", [P, KD, HD], BF16, kind="ExternalInput")
    wo_e = nc.dram_tensor("wo", [P, KD, SB], BF16, kind="ExternalInput")
    ckT_e = nc.dram_tensor("ckT", [HD, T_CACHE], BF16, kind="ExternalInput")
    cv_e = nc.dram_tensor("cv", [P, NJ_CACHE, HD], BF16, kind="ExternalInput")
    ropes_e = nc.dram_tensor("ropes", [HD, S], F32, kind="ExternalInput")
    ropep_e = nc.dram_tensor("ropep", [HD, S], F32, kind="ExternalInput")
    maskT_e = nc.dram_tensor("maskT", [P, NB, n_mask_chunks, SB], BF16,
                             kind="ExternalInput")
    out_e = nc.dram_tensor("out", [S, SB], F32, kind="ExternalOutput")

    with tile.TileContext(nc) as tc:
        with (
            tc.tile_pool(name="persist", bufs=1) as persist,
            tc.tile_pool(name="dram", bufs=1, space="DRAM") as dram,
        ):
            keysT = persist.tile([P, T], BF16)
            vals = persist.tile([P, NJ, HD + 1], BF16)
            qT = persist.tile([P, NH, S], BF16)
            ident = persist.tile([P, P], BF16)
            warm = persist.tile([P, 1], BF16)

            attT_my = [
                dram.tile([P, NH * SB], BF16, name=f"agin{b}")
                for b in range(NB - 1)
            ]
            attT_all = [
                dram.tile([N_CORES * P, NH * SB], BF16, name=f"agout{b}",
                          addr_space="Shared")
                for b in range(NB - 1)
            ]
            # first and last blocks gather in two head-halves (earlier
            # first-half availability / shorter final exposure)
            attT_myh = {
                (b, i): dram.tile([P, 2 * SB], BF16, name=f"aginh{b}_{i}")
                for b in (0, NB - 1) for i in range(2)
            }
            attT_allh = {
                (b, i): dram.tile([N_CORES * P, 2 * SB], BF16,
                                  name=f"agouth{b}_{i}", addr_space="Shared")
                for b in (0, NB - 1) for i in range(2)
            }

            # skew sponge: a tiny AllGather issued first thing. Collectives
            # on the ring serialize, and per-core launch skew (observed up
            # to ~70us) is absorbed by whichever collective runs first; this
            # one completes under stage A's projections instead of putting
            # that skew on the first attT gather's critical path.
            sponge_in = dram.tile([1, 16], BF16, name="sponge_in")
            sponge_out = dram.tile([N_CORES, 16], BF16, name="sponge_out",
                                   addr_space="Shared")
            nc.gpsimd.collective_compute(
                "AllGather",
                mybir.AluOpType.bypass,
                replica_groups=[list(range(N_CORES))],
                ins=[sponge_in.opt()],
                outs=[sponge_out.opt()],
            )

            nc.gpsimd.memset(ident, 0.0)
            nc.gpsimd.affine_select(
                out=ident, in_=ident, compare_op=mybir.AluOpType.not_equal,
                fill=1.0, base=0, pattern=[[-1, P]], channel_multiplier=1,
            )
            # pre-warm the ACT exp table set during stage A
            nc.scalar.activation(warm, ident[:, 0:1],
                                 mybir.ActivationFunctionType.Exp)
            nc.vector.memset(vals[:, :, HD : HD + 1], 1.0)

            # ---- stage A: projections ----
            with (
                tc.tile_pool(name="wA", bufs=1) as wA,
                tc.tile_pool(name="xt", bufs=2) as xtp,
                tc.tile_pool(name="psA", bufs=6, space="PSUM") as psA,
            ):
                wq_sb = wA.tile([P, KD, NH * HD], BF16)
                wk_sb = wA.tile([P, KD, HD], BF16)
                wv_sb = wA.tile([P, KD, HD], BF16)
                ropes_sb = wA.tile([P, S], F32)
                ropep_sb = wA.tile([P, S], F32)

                # first-needed chunks first: the k=0 weight piece and the
                # first xt chunks gate the very first matmul
                xt0 = xtp.tile([P, KD, SB], BF16, tag="xt", name="xt0")
                # k-granular first group so the very first matmul's inputs
                # arrive as early as possible
                for k in range(4):
                    nc.sync.dma_start(wq_sb[:, ts(k, 1), :],
                                      wq_e.ap()[:, ts(k, 1), :])
                    nc.sync.dma_start(xt0[:, ts(k, 1), :],
                                      xT_e.ap()[:, 0, ts(k, 1), :])
                for g in range(1, 8):
                    nc.sync.dma_start(wq_sb[:, ts(g, 4), :],
                                      wq_e.ap()[:, ts(g, 4), :])
                    nc.sync.dma_start(xt0[:, ts(g, 4), :],
                                      xT_e.ap()[:, 0, ts(g, 4), :])
                for g in range(4):
                    nc.sync.dma_start(wk_sb[:, ts(g, 8), :],
                                      wk_e.ap()[:, ts(g, 8), :])
                    nc.sync.dma_start(wv_sb[:, ts(g, 8), :],
                                      wv_e.ap()[:, ts(g, 8), :])
                nc.sync.dma_start(ropes_sb, ropes_e.ap())
                nc.sync.dma_start(ropep_sb, ropep_e.ap())
                # cache tensors gate stage-B start (~220us). Issued here they
                # finish by ~150us; pushing them later makes the wo load
                # collide with the first AllGather's transfer window, which
                # measurably doubles that collective's latency.
                nc.sync.dma_start(vals[:, 0:NJ_CACHE, 0:HD], cv_e.ap())
                nc.sync.dma_start(keysT[:, 0:T_CACHE], ckT_e.ap())

                for b in range(NB):
                    if b == 0:
                        xt = xt0
                    else:
                        xt = xtp.tile([P, KD, SB], BF16, tag="xt",
                                      name=f"xt{b}")
                        for g in range(4):
                            nc.sync.dma_start(xt[:, ts(g, 8), :],
                                              xT_e.ap()[:, b, ts(g, 8), :])
                    psq = [
                        psA.tile([P, SB], F32, tag="psA", name=f"psq{m}")
                        for m in range(NH)
                    ]
                    for k in range(KD):
                        for m in range(NH):
                            nc.tensor.matmul(
                                psq[m], wq_sb[:, k, ts(m, HD)], xt[:, k, :],
                                start=(k == 0), stop=(k == KD - 1),
                            )
                    for m in range(NH):
                        nc.vector.tensor_mul(
                            out=qT[:, m, ds(b * SB, SB)], in0=psq[m],
                            in1=ropes_sb[:, ds(b * SB, SB)],
                        )
                    psk = psA.tile([P, SB], F32, tag="psA")
                    for k in range(KD):
                        nc.tensor.matmul(
                            psk, wk_sb[:, k, :], xt[:, k, :],
                            start=(k == 0), stop=(k == KD - 1),
                        )
                    nc.vector.tensor_mul(
                        out=keysT[:, ds(T_CACHE + b * SB, SB)], in0=psk,
                        in1=ropep_sb[:, ds(b * SB, SB)],
                    )
                    for m in range(NB):
                        psv = psA.tile([P, SB], F32, tag="psA")
                        for k in range(KD):
                            nc.tensor.matmul(
                                psv[:, 0:HD], xt[:, k, ts(m, P)], wv_sb[:, k, :],
                                start=(k == 0), stop=(k == KD - 1),
                            )
                        nc.any.tensor_copy(
                            vals[:, NJ_CACHE + NB * b + m, 0:HD], psv[:, 0:HD]
                        )
            # ---- stages B+C interleaved ----
            with tc.tile_pool(name="woP", bufs=1) as woP:
                wo_sb = woP.tile([P, KD, SB], BF16)

                with (
                    tc.tile_pool(name="maskp", bufs=2) as maskp,
                    tc.tile_pool(name="ptp", bufs=4) as ptp,
                    tc.tile_pool(name="attsb", bufs=2) as attp,
                    tc.tile_pool(name="attL", bufs=2) as attL,
                    tc.tile_pool(name="small", bufs=8) as small,
                    tc.tile_pool(name="outp", bufs=2) as outp,
                    tc.tile_pool(name="psS", bufs=2, space="PSUM") as psS,
                    tc.tile_pool(name="psPV", bufs=2, space="PSUM") as psPV,
                    tc.tile_pool(name="psTr", bufs=1, space="PSUM") as psTr,
                    tc.tile_pool(name="psO", bufs=1, space="PSUM") as psO,
                ):
                    al_tiles = {}

                    # --- stage C work queue: one wo-matmul at a time ---
                    c_state = {"queue": [], "cur": None, "k": 0,
                               "drain": False, "alt": 0}
                    # blocks whose attT arrives in two head-half gathers
                    # accumulate their wo matmuls first-half-heads first
                    k_last = [4 * c + l for l in (0, 1) for c in range(N_CORES)]
                    k_last += [4 * c + l for l in (2, 3) for c in range(N_CORES)]

                    # queue items are (b, m, lo, hi): positions [lo, hi) of
                    # the block's k-order. The last block's chunks split in
                    # two halves so half0 (whose AllGather lands mid-block)
                    # can be absorbed during h3; the halves meet via an
                    # SBUF partial sum.
                    partials = {}

                    def c_open(item):
                        b, m, lo, hi = item
                        # outside attention (drain), alternate accumulators
                        # between the psO bank and a (now idle) psS bank so
                        # consecutive chunks overlap
                        c_state["alt"] += 1
                        if c_state["drain"] and (c_state["alt"] % 2 == 1):
                            po = psS.tile([P, 2, SB], F32, tag="psS",
                                          name=f"po{b}_{m}_{lo}")[:, 0, :]
                        else:
                            po = psO.tile([P, SB], F32, tag="psO",
                                          name=f"po{b}_{m}_{lo}")
                        c_state["cur"] = (b, m, lo, hi, po)
                        c_state["k"] = lo

                    def c_step(anchor=None):
                        """Emit one pending stage-C matmul, if any. `anchor`
                        pins the matmul's schedule position: the Tile
                        scheduler's cost model treats collectives as instant
                        and would otherwise hoist these wo-matmuls (which
                        transitively wait on an AllGather) into earlier
                        blocks, hard-stalling the in-order PE stream."""
                        if c_state["cur"] is None:
                            if not c_state["queue"]:
                                return False
                            c_open(c_state["queue"].pop(0))
                        b, m, lo, hi, po = c_state["cur"]
                        ki = c_state["k"]
                        k = k_last[ki] if b == NB - 1 else ki
                        cmm = nc.tensor.matmul(
                            po, al_tiles[b][:, k, ts(m, P)], wo_sb[:, k, :],
                            start=(ki == lo), stop=(ki == hi - 1),
                        )
                        if anchor is not None:
                            tile.add_dep_helper(cmm.ins, anchor,
                                                False, "pin C-mm to block")
                        c_state["k"] = ki + 1
                        if c_state["k"] == hi:
                            if lo == 0 and hi < KD:
                                # first half: stash the partial sum in SBUF
                                part = outp.tile([P, SB], F32, tag="part",
                                                 name=f"part{m}", bufs=4)
                                nc.vector.tensor_copy(part, po)
                                partials[(b, m)] = part
                            else:
                                ot = outp.tile([P, SB], F32, tag="outp")
                                if lo == 0:
                                    nc.vector.tensor_copy(ot, po)
                                else:
                                    nc.vector.tensor_add(
                                        out=ot, in0=partials.pop((b, m)),
                                        in1=po,
                                    )
                                nc.scalar.dma_start(
                                    out_e.ap().rearrange(
                                        "(r p) n -> p r n", p=P
                                    )[:, NB * b + m, :],
                                    ot,
                                )
                            c_state["cur"] = None
                        return True

                    def c_drain():
                        while c_step():
                            pass

                    def emit_half_ag(b, attT_sb, half):
                        """Gather heads [2*half, 2*half+2) of block b from
                        all cores, landing them at their global k-chunk
                        positions in the block's al tile."""
                        nc.gpsimd.dma_start(
                            attT_myh[b, half].rearrange("p (h s) -> p h s",
                                                        s=SB),
                            attT_sb[:, 2 * half : 2 * half + 2, :],
                        )
                        nc.gpsimd.collective_compute(
                            "AllGather",
                            mybir.AluOpType.bypass,
                            replica_groups=[list(range(N_CORES))],
                            ins=[attT_myh[b, half].opt()],
                            outs=[attT_allh[b, half].opt()],
                        )
                        if b not in al_tiles:
                            al_tiles[b] = attL.tile(
                                [P, KD, SB], BF16, tag="attL", name=f"alh{b}"
                            )
                        alb = al_tiles[b]
                        allh = attT_allh[b, half].rearrange(
                            "(c p) n -> p c n", p=P
                        )
                        for c in range(N_CORES):
                            nc.gpsimd.dma_start(
                                alb[:, ds(4 * c + 2 * half, 2), :],
                                allh[:, c, :].rearrange("p (l s) -> p l s",
                                                        s=SB),
                            )

                    mask_tiles = {}
                    ag_pending = {}

                    def load_mask(b):
                        nmask = max(0, nj_active[b] - mask_from)
                        if nmask == 0:
                            return
                        mt = maskp.tile([P, n_mask_chunks, SB], BF16,
                                        tag="mt", name=f"mt{b}")
                        nc.sync.dma_start(
                            mt[:, 0:nmask, :],
                            maskT_e.ap()[:, b, 0:nmask, :],
                        )
                        mask_tiles[b] = mt

                    load_mask(0)
                    # wo is needed only once stage-C matmuls start (mid
                    # block 1); issue after the more urgent mask tile
                    for g in range(8):
                        nc.sync.dma_start(wo_sb[:, ts(g, 4), :],
                                          wo_e.ap()[:, ts(g, 4), :])

                    def load_al(bb):
                        al = attL.tile([P, KD, SB], BF16, tag="attL",
                                       name=f"al{bb}")
                        allr = attT_all[bb].rearrange("(c p) n -> p c n", p=P)
                        for c in range(N_CORES):
                            nc.gpsimd.dma_start(
                                al[:, ts(c, NB), :],
                                allr[:, c, :].rearrange("p (h s) -> p h s",
                                                        s=SB),
                            )
                        al_tiles[bb] = al

                    # PSUM start=True clears has_written for the WHOLE bank
                    # (probe-verified), and two PV accumulators share each
                    # psPV bank. The Tile scheduler may freely reorder
                    # writes to disjoint regions, so the bank-clearing
                    # matmul needs explicit scheduling deps: it must come
                    # after the previous h-iteration's partner-region
                    # matmuls, and the partner's first (start=False) write
                    # must come after the clear.
                    pv_prev_last = {}

                    for b in range(NB):
                        nj = nj_active[b]
                        np_pairs = nj // 2
                        # masked (diagonal) pairs FIRST: their DVE mask
                        # multiplies then run at the start of each h when
                        # DVE is idle, instead of colliding with the
                        # normalize chain that gates the next h's PV
                        # accumulators
                        if np_pairs > 2:
                            po_list = [np_pairs - 2, np_pairs - 1]
                            po_list += list(range(np_pairs - 2))
                        else:
                            po_list = list(range(np_pairs))
                        ord_chunks = [c for p in po_list
                                      for c in (2 * p, 2 * p + 1)]
                        first_emit = {}
                        last_emit = {}
                        for m in range(NB):
                            live = [j for j in ord_chunks
                                    if (j, m) not in skip_map[b]]
                            first_emit[m] = live[0]
                            last_emit[m] = live[-1]
                        mt = mask_tiles.get(b)
                        last = b == NB - 1
                        attT_sb = attp.tile([P, NH, SB], BF16, tag="attsb")
                        for h in range(NH):
                            cur_pt = {}
                            pv01 = psPV.tile([P, 2, 130], F32, tag="psPV",
                                             name="pv01")
                            pv23 = psPV.tile([P, 2, 130], F32, tag="psPV",
                                             name="pv23")
                            pv_slice = [pv01[:, 0, 0:129], pv01[:, 1, 0:129],
                                        pv23[:, 0, 0:129], pv23[:, 1, 0:129]]
                            bank_clear = [None, None]  # per psPV bank
                            pv_last = {}

                            def emit_pv(j):
                                for m in range(NB):
                                    if (j, m) in skip_map[b]:
                                        continue
                                    st = False
                                    bank = m // 2
                                    if j == first_emit[m]:
                                        st = bank_clear[bank] is None
                                    mm = nc.tensor.matmul(
                                        pv_slice[m], cur_pt[j][:, ts(m, P)],
                                        vals[:, j, :],
                                        start=st, stop=(j == last_emit[m]),
                                    )
                                    if st:
                                        bank_clear[bank] = mm
                                        # the clear must not hoist between
                                        # the previous iteration's same-bank
                                        # matmuls
                                        for pm in (2 * bank, 2 * bank + 1):
                                            pl = pv_prev_last.get(pm)
                                            if pl is not None:
                                                tile.add_dep_helper(
                                                    mm.ins, pl.ins, False,
                                                    "bank clear after prev")
                                    elif j == first_emit[m]:
                                        # first write of the bank's partner:
                                        # must land after the bank clear
                                        tile.add_dep_helper(
                                            mm.ins, bank_clear[bank].ins,
                                            False, "first write after clear")
                                    pv_last[m] = mm

                            def emit_pv_pair(p):
                                emit_pv(2 * p)
                                emit_pv(2 * p + 1)

                            # arm each block's wo work only with a worst-case
                            # AllGather latency budget (observed jitter
                            # 17-72us). Interleaved wo only buys back the
                            # ACT-pacing slack (~93ns/pair); everything else
                            # relocates to the stall-free drain at the end,
                            # whose depth also hides the final collectives.
                            if b == 2 and h == 1:
                                c_state["queue"].extend(
                                    (0, m, 0, KD) for m in range(NB)
                                )
                            if b == 3 and h == 0:
                                c_state["queue"].extend(
                                    (1, m, 0, KD) for m in range(NB)
                                )
                            if b == 3 and h == 3:
                                c_state["queue"].extend(
                                    (2, m, 0, KD) for m in range(NB)
                                )
                            for idx, p in enumerate(po_list):
                                j0, j1 = 2 * p, 2 * p + 1
                                ps = psS.tile([P, 2, SB], F32, tag="psS")
                                smm = None
                                for jj, j in enumerate((j0, j1)):
                                    slo = s_lo[b].get(j, 0)
                                    if b == 0 and h == 0:
                                        # PSUM here may still hold unbounded
                                        # initial garbage whose exp overflows;
                                        # full-width scores overwrite it
                                        slo = 0
                                    mm = nc.tensor.matmul(
                                        ps[:, jj, ds(slo, SB - slo)],
                                        keysT[:, ts(j, P)],
                                        qT[:, h, ds(b * SB + slo, SB - slo)],
                                        start=True, stop=True,
                                    )
                                    if smm is None:
                                        smm = mm
                                pt = ptp.tile([P, 2, SB], BF16, tag="pt")
                                cur_pt[j0] = pt[:, 0, :]
                                cur_pt[j1] = pt[:, 1, :]
                                nc.scalar.activation(
                                    pt, ps, mybir.ActivationFunctionType.Exp
                                )
                                for j in (j0, j1):
                                    if j >= mask_from and j in mul_map[b]:
                                        nc.vector.tensor_mul(
                                            out=cur_pt[j], in0=cur_pt[j],
                                            in1=mt[:, j - mask_from, :],
                                        )
                                # two-pair lag: the first PV of this h lands
                                # two pairs in, giving the previous h's
                                # normalize chain time to release the shared
                                # PV banks
                                if idx >= 2:
                                    emit_pv_pair(po_list[idx - 2])
                                n_pace = (1, 1, 2, 2)[b]
                                for _ in range(n_pace):
                                    c_step(smm.ins)
                            for tp in po_list[-2:]:
                                emit_pv_pair(tp)
                            pv_prev_last = pv_last

                            for m in range(NB):
                                rc = small.tile([P, 1], F32, tag="rc")
                                nc.vector.reciprocal(
                                    rc, pv_slice[m][:, HD : HD + 1]
                                )
                                at = small.tile([P, P], BF16, tag="at")
                                nc.vector.tensor_scalar_mul(
                                    at, pv_slice[m][:, 0:HD], rc
                                )
                                ptr = psTr.tile([P, P], F32, tag="ptr")
                                # transpose as a REGULAR matmul (at.T @ I):
                                # is_transpose matmuls are serialized against
                                # collectives by Tile, which would stall each
                                # block's normalize on the previous AllGather
                                nc.tensor.matmul(ptr, at, ident,
                                                 start=True, stop=True)
                                nc.vector.tensor_copy(
                                    attT_sb[:, h, ts(m, P)], ptr
                                )

                            # last block: gather each half of the heads as
                            # soon as it is done, shortening the final AG
                            # exposure
                            if last and h == 1:
                                emit_half_ag(b, attT_sb, 0)
                            if last and h == 3:
                                emit_half_ag(b, attT_sb, 1)
                                c_state["queue"].extend(
                                    (NB - 1, m, 0, KD // 2) for m in range(NB)
                                )
                                c_state["queue"].extend(
                                    (NB - 1, m, KD // 2, KD)
                                    for m in range(NB)
                                )
                            # load the previous block's gathered attT as
                            # early as possible (the unpack DMAs just wait
                            # on the collective in the gpsimd queue, which
                            # has nothing else to do mid-block)
                            if h == 0 and b - 1 in ag_pending:
                                load_al(ag_pending.pop(b - 1))

                        if b + 1 < NB:
                            load_mask(b + 1)
                        if not last:
                            nc.gpsimd.dma_start(
                                attT_my[b].rearrange("p (h s) -> p h s", s=SB),
                                attT_sb,
                            )
                            nc.gpsimd.collective_compute(
                                "AllGather",
                                mybir.AluOpType.bypass,
                                replica_groups=[list(range(N_CORES))],
                                ins=[attT_my[b].opt()],
                                outs=[attT_all[b].opt()],
                            )
                            ag_pending[b] = b

                    c_state["drain"] = True
                    c_drain()

    nc.compile()
    _BUILD_CACHE[key] = nc
    return nc


def _prep_inputs(x, rope, mask, cache_k, cache_v, wq, wk, wv, wo):
    """Host-side shard + partition-major layout prep (every DMA becomes a
    single contiguous run per partition)."""
    scale = np.float32(1.0 / np.sqrt(HD))
    x2 = np.ascontiguousarray(np.asarray(x).reshape(S, D), dtype=np.float32)
    xT = x2.T.astype(NPBF16)                       # [D, S]
    xTh = np.ascontiguousarray(
        xT.reshape(KD, P, NB, SB).transpose(1, 2, 0, 3)
    )                                              # [P, NB, KD, SB]
    rope2 = np.asarray(rope).reshape(S, HD).astype(np.float32)
    ropesT = np.ascontiguousarray((rope2 * scale).T)
    ropepT = np.ascontiguousarray(rope2.T)

    m2 = np.asarray(mask).reshape(S, T).astype(np.float32)
    cache_zero = bool(np.all(m2[:, :T_CACHE] == 0.0))
    causal = m2[:, T_CACHE:]
    # s-block b may skip t-chunk j (j >= 16) iff every entry of the
    # (s-block, chunk) tile is <= -1e3 (exp underflows to ~0 exactly as in
    # the reference softmax).
    nj_active = []
    for b in range(NB):
        nj = NJ
        for j in range(NJ - 1, NJ_CACHE - 1, -1):
            blk = causal[
                b * SB : (b + 1) * SB,
                (j - NJ_CACHE) * 128 : (j - NJ_CACHE + 1) * 128,
            ]
            if np.all(blk <= -1e3):
                nj = j
            else:
                break
        nj_active.append(nj)

    if cache_zero:
        mask_from = NJ_CACHE
        mask_used = causal
    else:
        mask_from = 0
        nj_active = [NJ] * NB
        mask_used = m2
    # keep pairs of t-chunks intact for the paired exp
    nj_active = [min(NJ, nj + (nj % 2)) for nj in nj_active]
    # multiplicative form: probs = exp(scores) * exp(mask)
    mask01 = np.exp(mask_used.astype(np.float64)).astype(NPBF16)
    maskT = np.ascontiguousarray(mask01.T)         # [mask_rows, S]
    mask_rows = maskT.shape[0]
    n_mask_chunks = mask_rows // 128
    maskTh = np.ascontiguousarray(
        maskT.reshape(n_mask_chunks, P, NB, SB).transpose(1, 2, 0, 3)
    )                                              # [P, NB, n_chunks, SB]

    # PV sub-matmul skip map and mask-multiply map, from the actual mask
    skip_map = []
    mul_map = []
    for b in range(NB):
        skips = set()
        muls = set()
        for j in range(mask_from, nj_active[b]):
            # mtile: [128 t, 512 s] (maskT layout: rows=t, cols=s)
            mtile = maskT[
                (j - mask_from) * 128 : (j - mask_from + 1) * 128,
                b * SB : (b + 1) * SB,
            ]
            if not np.all(mtile == np.float32(1.0)):
                muls.add(j)
            for m in range(NB):
                sub = mtile[:, m * P : (m + 1) * P]
                if np.all(sub == np.float32(0.0)):
                    skips.add((j, m))
        skip_map.append(frozenset(skips))
        mul_map.append(frozenset(muls))

    wq_n = np.asarray(wq)
    wk_n = np.asarray(wk)
    wv_n = np.asarray(wv)
    wo_n = np.asarray(wo)
    ck_n = np.asarray(cache_k)
    cv_n = np.asarray(cache_v)

    def pmajor(w, ncols):  # [D, ncols] -> [P, KD, ncols]
        return np.ascontiguousarray(
            w.astype(NPBF16).reshape(KD, P, ncols).transpose(1, 0, 2)
        )

    in_maps = []
    for c in range(N_CORES):
        cvp = np.ascontiguousarray(
            cv_n[0, :, c, :].astype(NPBF16)
            .reshape(NJ_CACHE, P, HD).transpose(1, 0, 2)
        )                                          # [P, NJ_CACHE, HD]
        in_maps.append({
            "xT": xTh,
            "wq": pmajor(wq_n[:, c * NH * HD : (c + 1) * NH * HD], NH * HD),
            "wk": pmajor(wk_n[:, c * HD : (c + 1) * HD], HD),
            "wv": pmajor(wv_n[:, c * HD : (c + 1) * HD], HD),
            "wo": pmajor(wo_n[:, c * SB : (c + 1) * SB], SB),
            "ckT": np.ascontiguousarray(ck_n[0, :, c, :].T).astype(NPBF16),
            "cv": cvp,
            "ropes": ropesT,
            "ropep": ropepT,
            "maskT": maskTh,
        })
    return in_maps, nj_active, mask_from, mask_rows, skip_map, mul_map


def kernel_impl(inputs, trace=False, tmpdir=None):
    in_maps, nj_active, mask_from, mask_rows, skip_map, mul_map = \
        _prep_inputs(**inputs)
    nc = build_kernel(nj_active, mask_from, mask_rows, skip_map, mul_map)
    res = run_bass_kernel_spmd(
        nc, in_maps, core_ids=list(range(N_CORES)), trace=trace, tmpdir=tmpdir
    )
    out = np.concatenate(
        [res.results[c]["out"] for c in range(N_CORES)], axis=1
    ).reshape(1, S, H * HD)
    return np.ascontiguousarray(out, dtype=np.float32), res


def kernel(**inputs) -> np.ndarray:
    out, _ = kernel_impl(inputs, trace=False)
    return out
